# revision 1
# baseline (speedup 1.0000x reference)
"""GRU cell kernel for Trainium2, data-parallel over 8 NeuronCores.

Reference computation (B=4096, I=H=2048, C=I+H=4096):
    combined   = [x, h]                                   [B, C]
    to_update  = sigmoid(combined @ W_update.T + b_u)     [B, H]
    to_select  = sigmoid(combined @ W_select.T + b_s)     [B, H]
    updated    = h * to_update
    new_comb   = [x, updated]
    predictions= tanh(new_comb @ W_predict.T + b_p)
    h_new      = h * (1 - to_select) + predictions * to_select

Sharding: batch split 8 ways (512 rows/core), weights replicated.
On-chip layout is [feature, batch] (transposed), so each weight tile
[128c, 128h] is the stationary matmul operand and activation tiles
[128c, 512b] are the moving operand -- no on-chip transposes anywhere.
Matmuls run in bf16 (inputs host-cast) with fp32 PSUM accumulation;
gates and the final blend stay fp32.
"""

from contextlib import ExitStack

import numpy as np
import ml_dtypes

import concourse.bass as bass
import concourse.tile as tile
import concourse.mybir as mybir
from concourse import bacc
from concourse.bass_utils import run_bass_kernel_spmd

BF16 = mybir.dt.bfloat16
F32 = mybir.dt.float32
NPBF16 = ml_dtypes.bfloat16

B, I, H = 4096, 2048, 2048
C = I + H
NCORES = 8
BS = B // NCORES            # 512 batch rows per core
P = 128                     # SBUF partitions
HT = H // P                 # 16 output-row tiles
IT = I // P                 # 16 x feature tiles
CT = C // P                 # 32 contraction tiles
ACT_F = mybir.ActivationFunctionType

_PROGRAM = None


def _build_program():
    nc = bacc.Bacc("TRN2")

    xT = nc.dram_tensor("xT", [IT, P, BS], BF16, kind="ExternalInput")
    hT32 = nc.dram_tensor("hT32", [HT, P, BS], F32, kind="ExternalInput")
    Wu = nc.dram_tensor("Wu", [HT, P, C], BF16, kind="ExternalInput")
    Ws = nc.dram_tensor("Ws", [HT, P, C], BF16, kind="ExternalInput")
    Wp = nc.dram_tensor("Wp", [HT, P, C], BF16, kind="ExternalInput")
    bu = nc.dram_tensor("bu", [P, HT], F32, kind="ExternalInput")
    bsel = nc.dram_tensor("bsel", [P, HT], F32, kind="ExternalInput")
    bp = nc.dram_tensor("bp", [P, HT], F32, kind="ExternalInput")
    out = nc.dram_tensor("out", [HT, P, BS], F32, kind="ExternalOutput")

    with tile.TileContext(nc) as tc, ExitStack() as ctx:
        singles = ctx.enter_context(tc.tile_pool(name="singles", bufs=1))
        wpool = ctx.enter_context(tc.tile_pool(name="wpool", bufs=4))
        pspool = ctx.enter_context(tc.tile_pool(name="ps", bufs=8, space="PSUM"))
        work = ctx.enter_context(tc.tile_pool(name="work", bufs=4))

        bu_sb = singles.tile([P, HT], F32, name="bu_sb")
        nc.sync.dma_start(bu_sb[:], bu[:])
        bs_sb = singles.tile([P, HT], F32, name="bs_sb")
        nc.sync.dma_start(bs_sb[:], bsel[:])
        bp_sb = singles.tile([P, HT], F32, name="bp_sb")
        nc.sync.dma_start(bp_sb[:], bp[:])

        # combined.T tiles: 16 x-tiles then 16 h-tiles (all bf16 [128, 512])
        comb = []
        for n in range(IT):
            t = singles.tile([P, BS], BF16, name=f"combx{n}", tag=f"combx{n}")
            nc.sync.dma_start(t[:], xT[n])
            comb.append(t)
        # h arrives fp32 (needed for the final blend); bf16 copies are cast
        # on-chip to avoid a second HBM upload of h.
        h32 = []
        for i in range(HT):
            t = singles.tile([P, BS], F32, name=f"h32_{i}", tag=f"h32_{i}")
            nc.sync.dma_start(t[:], hT32[i])
            h32.append(t)
        for i in range(HT):
            t = singles.tile([P, BS], BF16, name=f"combh{i}", tag=f"combh{i}")
            nc.vector.tensor_copy(t[:], h32[i][:])
            comb.append(t)

        upd = [
            singles.tile([P, BS], BF16, name=f"upd{i}", tag=f"upd{i}")
            for i in range(HT)
        ]
        sel = [
            singles.tile([P, BS], F32, name=f"sel{i}", tag=f"sel{i}")
            for i in range(HT)
        ]

        def gemm(W, rhs_tiles, i):
            """psum[128h, 512b] = sum_c W_tile[i].T @ rhs  (bf16, fp32 accum)"""
            wblk = wpool.tile([P, C], BF16, tag="wblk", name="wblk")
            half = C // 2
            nc.sync.dma_start(wblk[:, 0:half], W[i, :, 0:half])
            nc.sync.dma_start(wblk[:, half:C], W[i, :, half:C])
            ps = pspool.tile([P, BS], F32, tag="ps", name="ps")
            for n in range(CT):
                nc.tensor.matmul(
                    ps,
                    wblk[:, n * P:(n + 1) * P],
                    rhs_tiles[n],
                    start=(n == 0),
                    stop=(n == CT - 1),
                )
            return ps

        # update gate -> updated = h * sigmoid(z_u)  (bf16, feeds matmul 3)
        for i in range(HT):
            ps = gemm(Wu, comb, i)
            u = work.tile([P, BS], BF16, tag="u", name="u")
            nc.scalar.activation(u[:], ps[:], ACT_F.Sigmoid, bias=bu_sb[:, i:i + 1])
            nc.vector.tensor_mul(upd[i][:], comb[IT + i][:], u[:])

        # select gate (fp32, used in final blend)
        for i in range(HT):
            ps = gemm(Ws, comb, i)
            nc.scalar.activation(
                sel[i][:], ps[:], ACT_F.Sigmoid, bias=bs_sb[:, i:i + 1]
            )

        # predictions + blend: h_new = h + sel * (tanh(z_p) - h)
        newcomb = comb[:IT] + upd
        for i in range(HT):
            ps = gemm(Wp, newcomb, i)
            p_t = work.tile([P, BS], F32, tag="p", name="p_t")
            nc.scalar.activation(p_t[:], ps[:], ACT_F.Tanh, bias=bp_sb[:, i:i + 1])
            d = work.tile([P, BS], F32, tag="d", name="d")
            nc.vector.tensor_sub(d[:], p_t[:], h32[i][:])
            nc.vector.tensor_mul(d[:], d[:], sel[i][:])
            o = work.tile([P, BS], F32, tag="o", name="o")
            nc.vector.tensor_add(o[:], h32[i][:], d[:])
            nc.sync.dma_start(out[i], o[:])

    nc.finalize()
    return nc


def _get_program():
    global _PROGRAM
    if _PROGRAM is None:
        _PROGRAM = _build_program()
    return _PROGRAM


def _pack_weight(w):
    """[H, C] fp32 -> [HT, P, C] bf16 with [i, p, n*128+m] = W[i*128+m, n*128+p].

    Slice [i] is then an SBUF block whose column window n*128:(n+1)*128 is the
    stationary operand (lhsT = W.T tile) for contraction tile n.
    """
    wb = np.asarray(w, dtype=np.float32).astype(NPBF16)
    return np.ascontiguousarray(
        wb.reshape(HT, P, CT, P).transpose(0, 3, 2, 1).reshape(HT, P, C)
    )


def _prep_inputs(x, h, W_update, b_update, W_select, b_select, W_predict, b_predict):
    x = np.asarray(x, dtype=np.float32)
    h = np.asarray(h, dtype=np.float32)

    Wu = _pack_weight(W_update)
    Ws = _pack_weight(W_select)
    Wp = _pack_weight(W_predict)
    bu = np.ascontiguousarray(
        np.asarray(b_update, dtype=np.float32).reshape(HT, P).T
    )
    bsel = np.ascontiguousarray(
        np.asarray(b_select, dtype=np.float32).reshape(HT, P).T
    )
    bp = np.ascontiguousarray(
        np.asarray(b_predict, dtype=np.float32).reshape(HT, P).T
    )

    in_maps = []
    for c in range(NCORES):
        rows = slice(c * BS, (c + 1) * BS)
        xT = np.ascontiguousarray(x[rows].T.astype(NPBF16).reshape(IT, P, BS))
        hT32 = np.ascontiguousarray(h[rows].T.reshape(HT, P, BS))
        in_maps.append(
            {
                "xT": xT,
                "hT32": hT32,
                "Wu": Wu,
                "Ws": Ws,
                "Wp": Wp,
                "bu": bu,
                "bsel": bsel,
                "bp": bp,
            }
        )
    return in_maps


def kernel(x, h, W_update, b_update, W_select, b_select, W_predict, b_predict,
           _trace=False):
    nc = _get_program()
    in_maps = _prep_inputs(
        x, h, W_update, b_update, W_select, b_select, W_predict, b_predict
    )
    res = run_bass_kernel_spmd(
        nc, in_maps, core_ids=list(range(NCORES)), trace=_trace
    )
    h_new = np.empty((B, H), dtype=np.float32)
    for c in range(NCORES):
        rows = slice(c * BS, (c + 1) * BS)
        h_new[rows] = res.results[c]["out"].reshape(H, BS).T
    if _trace:
        return h_new, res
    return h_new



# revision 2
# speedup vs baseline: 1.3086x; 1.3086x over previous
"""GRU cell kernel for Trainium2, data-parallel over 8 NeuronCores.

Reference computation (B=4096, I=H=2048, C=I+H=4096):
    combined   = [x, h]                                   [B, C]
    to_update  = sigmoid(combined @ W_update.T + b_u)     [B, H]
    to_select  = sigmoid(combined @ W_select.T + b_s)     [B, H]
    updated    = h * to_update
    new_comb   = [x, updated]
    predictions= tanh(new_comb @ W_predict.T + b_p)
    h_new      = h * (1 - to_select) + predictions * to_select
             ( = h - h*to_select + predictions*to_select )

Sharding: batch split 8 ways (512 rows/core), weights replicated.
On-chip layout is [feature, batch] (transposed), so each weight tile
[128c, 128h] is the stationary matmul operand and activation tiles
[128c, 512b] are the moving operand -- no on-chip transposes anywhere.
Matmuls run in bf16 (inputs host-cast) with fp32 PSUM accumulation;
the final blend uses fp32 h.

Pipeline-latency structure (per core, cost-model driven):
  * 1536 matmuls x ~215 ns is the PE floor (~330 us); everything else is
    arranged to keep PE busy from ~1.5 us onward.
  * Prologue: h is uploaded twice (bf16 for matmuls, fp32 for the blend)
    so no on-chip cast sits on the critical path.  The first PRE update-
    gate gemms run their x-half contraction first (interleaved across
    PSUM banks, n-outer) so the PE starts as soon as x tile 0 + the
    first x-half weight block land, while the h tiles stream in behind.
  * h*(1-sel) is precomputed during the select phase (DVE slack), so the
    predict-phase tail per tile is act + 2 DVE ops + store; the last
    output tile is split into two half-batch PSUM groups to halve the
    exposed tail.
"""

from contextlib import ExitStack

import numpy as np
import ml_dtypes

import concourse.bass as bass
import concourse.tile as tile
import concourse.mybir as mybir
from concourse import bacc
from concourse.bass_utils import run_bass_kernel_spmd

BF16 = mybir.dt.bfloat16
F32 = mybir.dt.float32
NPBF16 = ml_dtypes.bfloat16

B, I, H = 4096, 2048, 2048
C = I + H
NCORES = 8
BS = B // NCORES            # 512 batch rows per core
P = 128                     # SBUF partitions
HT = H // P                 # 16 output-row tiles
IT = I // P                 # 16 x feature tiles
CT = C // P                 # 32 contraction tiles
HALF = C // 2
PRE = 4                     # update-gate gemms with split x/h contraction
ACT_F = mybir.ActivationFunctionType

_PROGRAM = None


def _build_program():
    nc = bacc.Bacc("TRN2")

    xT = nc.dram_tensor("xT", [IT, P, BS], BF16, kind="ExternalInput")
    hTb = nc.dram_tensor("hTb", [HT, P, BS], BF16, kind="ExternalInput")
    hT32 = nc.dram_tensor("hT32", [HT, P, BS], F32, kind="ExternalInput")
    Wu = nc.dram_tensor("Wu", [HT, P, C], BF16, kind="ExternalInput")
    Ws = nc.dram_tensor("Ws", [HT, P, C], BF16, kind="ExternalInput")
    Wp = nc.dram_tensor("Wp", [HT, P, C], BF16, kind="ExternalInput")
    bu = nc.dram_tensor("bu", [P, HT], F32, kind="ExternalInput")
    bsel = nc.dram_tensor("bsel", [P, HT], F32, kind="ExternalInput")
    bp = nc.dram_tensor("bp", [P, HT], F32, kind="ExternalInput")
    out = nc.dram_tensor("out", [HT, P, BS], F32, kind="ExternalOutput")

    with tile.TileContext(nc) as tc, ExitStack() as ctx:
        singles = ctx.enter_context(tc.tile_pool(name="singles", bufs=1))
        wpool = ctx.enter_context(tc.tile_pool(name="wpool", bufs=6))
        pspool = ctx.enter_context(tc.tile_pool(name="ps", bufs=8, space="PSUM"))
        work = ctx.enter_context(tc.tile_pool(name="work", bufs=4))

        bu_sb = singles.tile([P, HT], F32, name="bu_sb")
        nc.sync.dma_start(bu_sb[:], bu[:])
        bs_sb = singles.tile([P, HT], F32, name="bs_sb")
        nc.sync.dma_start(bs_sb[:], bsel[:])
        bp_sb = singles.tile([P, HT], F32, name="bp_sb")
        nc.sync.dma_start(bp_sb[:], bp[:])

        # combined.T tiles: 16 x-tiles then 16 h-tiles (all bf16 [128, 512]).
        # Issue x first: the prologue x-half gemms depend only on these.
        comb = []
        for n in range(IT):
            t = singles.tile([P, BS], BF16, name=f"combx{n}", tag=f"combx{n}")
            nc.sync.dma_start(t[:], xT[n])
            comb.append(t)

        # x-half weight blocks for the first PRE update gemms -- loaded
        # before the bf16 h tiles so the PE can start on them immediately.
        wxs = []
        for i in range(PRE):
            wx = wpool.tile([P, HALF], BF16, tag="wx", name="wx")
            nc.sync.dma_start(wx[:], Wu[i, :, 0:HALF])
            wxs.append(wx)

        for i in range(HT):
            t = singles.tile([P, BS], BF16, name=f"combh{i}", tag=f"combh{i}")
            nc.sync.dma_start(t[:], hTb[i])
            comb.append(t)

        upd = [
            singles.tile([P, BS], BF16, name=f"upd{i}", tag=f"upd{i}")
            for i in range(HT)
        ]
        selb = [
            singles.tile([P, BS], BF16, name=f"selb{i}", tag=f"selb{i}")
            for i in range(HT)
        ]
        hs = [
            singles.tile([P, BS], F32, name=f"hs{i}", tag=f"hs{i}")
            for i in range(HT)
        ]

        def load_w(W, i):
            wx = wpool.tile([P, HALF], BF16, tag="wx", name="wx")
            nc.sync.dma_start(wx[:], W[i, :, 0:HALF])
            wh = wpool.tile([P, HALF], BF16, tag="wh", name="wh")
            nc.sync.dma_start(wh[:], W[i, :, HALF:C])
            return wx, wh

        def mm_half(ps, w, rhs_tiles, n0, n1, start, stop, cols=None):
            for n in range(n0, n1):
                w_ap = w[:, (n - n0) * P:(n - n0 + 1) * P]
                r = rhs_tiles[n]
                nc.tensor.matmul(
                    ps,
                    w_ap,
                    r[:] if cols is None else r[:, cols],
                    start=(start and n == n0),
                    stop=(stop and n == n1 - 1),
                )

        # ---- update gate: upd[i] = h * sigmoid(z_u) ----
        # First PRE gemms: x-half contraction interleaved across PSUM banks
        # (n outer) so PE work starts before the h tiles arrive.
        psA = []
        for i in range(PRE):
            ps = pspool.tile([P, BS], F32, tag="ps", name="ps")
            psA.append(ps)
        for n in range(IT):
            for i in range(PRE):
                nc.tensor.matmul(
                    psA[i], wxs[i][:, n * P:(n + 1) * P], comb[n],
                    start=(n == 0), stop=False,
                )

        def finish_update(i, ps):
            u = work.tile([P, BS], BF16, tag="u", name="u")
            nc.scalar.activation(u[:], ps[:], ACT_F.Sigmoid, bias=bu_sb[:, i:i + 1])
            nc.vector.tensor_mul(upd[i][:], comb[IT + i][:], u[:])

        for i in range(PRE):
            wh = wpool.tile([P, HALF], BF16, tag="wh", name="wh")
            nc.sync.dma_start(wh[:], Wu[i, :, HALF:C])
            mm_half(psA[i], wh, comb, IT, CT, start=False, stop=True)
            finish_update(i, psA[i])

        for i in range(PRE, HT):
            wx, wh = load_w(Wu, i)
            ps = pspool.tile([P, BS], F32, tag="ps", name="ps")
            mm_half(ps, wx, comb, 0, IT, start=True, stop=False)
            mm_half(ps, wh, comb, IT, CT, start=False, stop=True)
            finish_update(i, ps)

        # fp32 h tiles: only needed from the select phase on (blend terms),
        # so their DMAs are issued after the update-phase loads.
        h32 = []
        for i in range(HT):
            t = singles.tile([P, BS], F32, name=f"h32_{i}", tag=f"h32_{i}")
            nc.sync.dma_start(t[:], hT32[i])
            h32.append(t)

        # ---- select gate ----
        # sel kept bf16 (only multiplies |tanh| <= 1); h*(1-sel) precomputed
        # here in fp32 where the big h magnitudes live.
        for i in range(HT):
            wx, wh = load_w(Ws, i)
            ps = pspool.tile([P, BS], F32, tag="ps", name="ps")
            mm_half(ps, wx, comb, 0, IT, start=True, stop=False)
            mm_half(ps, wh, comb, IT, CT, start=False, stop=True)
            s32 = work.tile([P, BS], F32, tag="s32", name="s32")
            nc.scalar.activation(s32[:], ps[:], ACT_F.Sigmoid, bias=bs_sb[:, i:i + 1])
            nc.vector.tensor_copy(selb[i][:], s32[:])
            hm = work.tile([P, BS], F32, tag="hm", name="hm")
            nc.vector.tensor_mul(hm[:], h32[i][:], s32[:])
            nc.vector.tensor_sub(hs[i][:], h32[i][:], hm[:])

        # ---- predictions + blend: h_new = hs + sel * tanh(z_p) ----
        newcomb = comb[:IT] + upd

        def blend(i, ps, cols):
            p_t = work.tile([P, BS // 2], F32, tag="p", name="p_t")
            nc.scalar.activation(
                p_t[:], ps[:, cols] if ps.shape[1:] == (BS,) else ps[:],
                ACT_F.Tanh, bias=bp_sb[:, i:i + 1],
            )
            o = work.tile([P, BS // 2], F32, tag="o", name="o")
            nc.vector.tensor_mul(o[:], p_t[:], selb[i][:, cols])
            nc.vector.tensor_add(o[:], o[:], hs[i][:, cols])
            nc.sync.dma_start(out[i, :, cols], o[:])

        for i in range(HT - 1):
            wx, wh = load_w(Wp, i)
            ps = pspool.tile([P, BS], F32, tag="ps", name="ps")
            mm_half(ps, wx, newcomb, 0, IT, start=True, stop=False)
            mm_half(ps, wh, newcomb, IT, CT, start=False, stop=True)
            blend(i, ps, slice(0, BS // 2))
            blend(i, ps, slice(BS // 2, BS))

        # Last tile: two half-batch accumulation groups so the first half's
        # act+blend+store overlaps the second half's matmuls.
        i = HT - 1
        wx, wh = load_w(Wp, i)
        for hcol in range(2):
            cols = slice(hcol * (BS // 2), (hcol + 1) * (BS // 2))
            ps = pspool.tile([P, BS // 2], F32, tag="pshalf", name="pshalf")
            mm_half(ps, wx, newcomb, 0, IT, start=True, stop=False, cols=cols)
            mm_half(ps, wh, newcomb, IT, CT, start=False, stop=True, cols=cols)
            blend(i, ps, cols)

    nc.finalize()
    return nc


def _get_program():
    global _PROGRAM
    if _PROGRAM is None:
        _PROGRAM = _build_program()
    return _PROGRAM


def _pack_weight(w):
    """[H, C] fp32 -> [HT, P, C] bf16 with [i, p, n*128+m] = W[i*128+m, n*128+p].

    Slice [i] is then an SBUF block whose column window n*128:(n+1)*128 is the
    stationary operand (lhsT = W.T tile) for contraction tile n.
    """
    wb = np.asarray(w, dtype=np.float32).astype(NPBF16)
    return np.ascontiguousarray(
        wb.reshape(HT, P, CT, P).transpose(0, 3, 2, 1).reshape(HT, P, C)
    )


def _prep_inputs(x, h, W_update, b_update, W_select, b_select, W_predict, b_predict):
    x = np.asarray(x, dtype=np.float32)
    h = np.asarray(h, dtype=np.float32)

    Wu = _pack_weight(W_update)
    Ws = _pack_weight(W_select)
    Wp = _pack_weight(W_predict)
    bu = np.ascontiguousarray(
        np.asarray(b_update, dtype=np.float32).reshape(HT, P).T
    )
    bsel = np.ascontiguousarray(
        np.asarray(b_select, dtype=np.float32).reshape(HT, P).T
    )
    bp = np.ascontiguousarray(
        np.asarray(b_predict, dtype=np.float32).reshape(HT, P).T
    )

    in_maps = []
    for c in range(NCORES):
        rows = slice(c * BS, (c + 1) * BS)
        xT = np.ascontiguousarray(x[rows].T.astype(NPBF16).reshape(IT, P, BS))
        hT32 = np.ascontiguousarray(h[rows].T.reshape(HT, P, BS))
        hTb = np.ascontiguousarray(hT32.astype(NPBF16))
        in_maps.append(
            {
                "xT": xT,
                "hTb": hTb,
                "hT32": hT32,
                "Wu": Wu,
                "Ws": Ws,
                "Wp": Wp,
                "bu": bu,
                "bsel": bsel,
                "bp": bp,
            }
        )
    return in_maps


def kernel(x, h, W_update, b_update, W_select, b_select, W_predict, b_predict,
           _trace=False):
    nc = _get_program()
    in_maps = _prep_inputs(
        x, h, W_update, b_update, W_select, b_select, W_predict, b_predict
    )
    res = run_bass_kernel_spmd(
        nc, in_maps, core_ids=list(range(NCORES)), trace=_trace
    )
    h_new = np.empty((B, H), dtype=np.float32)
    for c in range(NCORES):
        rows = slice(c * BS, (c + 1) * BS)
        h_new[rows] = res.results[c]["out"].reshape(H, BS).T
    if _trace:
        return h_new, res
    return h_new
